# revision 16
# baseline (speedup 1.0000x reference)
"""Trainium2 Bass kernel for a cross-modal transformer block (attention + FFN).

Contract: kernel(**inputs) takes the FULL unsharded inputs (numpy, fp32) and
returns the FULL output [4, 2048, 512] fp32.

Sharding: 8 cores = data-parallel over batch (4) x query-sequence halves (2).
Each core computes K/V projections for its batch's full 2048-token sequence
(cheap duplication) so attention needs no collectives.

Device layout: everything feature-major ([features on partitions, tokens on
free]); the host pre-transposes and pre-casts inputs so the device does zero
transposes.
"""

import functools
import sys

import numpy as np

sys.path.insert(0, "/opt/trn_rl_repo")

import ml_dtypes  # noqa: E402

import concourse.bass as bass  # noqa: E402
import concourse.tile as tile  # noqa: E402
from concourse import bacc, mybir  # noqa: E402
from concourse.bass_utils import run_bass_kernel_spmd  # noqa: E402

BF16 = mybir.dt.bfloat16
F32 = mybir.dt.float32
AF = mybir.ActivationFunctionType
OP = mybir.AluOpType

B, S, D = 4, 2048, 512
H, DH = 8, 64
FF = 2048
P = 128
C = D // P  # 4 feature chunks
CF = FF // P  # 16 ffn chunks
TQ = S // 2  # 1024 query tokens per core
TK = S  # full key sequence per core
KC = TK // P  # 16 key chunks
NT = 512  # token tile (matmul free dim)
NQ = TQ // NT  # 2 query-token tiles
SCALE = 1.0 / np.sqrt(DH)  # 0.125
LN_EPS = 1e-5
NCORES = 8


def _emit(nc, t, es, tc):
    """Emit the per-core program. t: dict name -> DRAM AP."""
    # ---------------- pools ----------------
    wp = es.enter_context(tc.tile_pool(name="w", bufs=1))
    ap_ = es.enter_context(tc.tile_pool(name="acts", bufs=1))
    ptq = es.enter_context(tc.tile_pool(name="ptq", bufs=1))
    psA = es.enter_context(tc.tile_pool(name="psA", bufs=2, space="PSUM"))
    psS = es.enter_context(tc.tile_pool(name="psS", bufs=2, space="PSUM"))
    psC = es.enter_context(tc.tile_pool(name="psC", bufs=2, space="PSUM"))
    stream = es.enter_context(tc.tile_pool(name="stream", bufs=6))
    wstream = es.enter_context(tc.tile_pool(name="wstream", bufs=4))
    stage = es.enter_context(tc.tile_pool(name="stage", bufs=1))
    chunk = es.enter_context(tc.tile_pool(name="chunk", bufs=2))
    small = es.enter_context(tc.tile_pool(name="small", bufs=8))
    epool = es.enter_context(tc.tile_pool(name="e", bufs=4))
    hpool = es.enter_context(tc.tile_pool(name="h", bufs=1))

    # ---------------- constants / weights ----------------
    def ld_w(name, kchunks, n):
        w = wp.tile([P, kchunks, n], BF16, name=name + "_sb")
        nc.sync.dma_start(w, t[name].rearrange("(c p) o -> p c o", p=P))
        return w

    def ld_b(name, kchunks):
        b = wp.tile([P, kchunks], F32, name=name + "_sb")
        nc.sync.dma_start(b, t[name].rearrange("(c p) -> p c", p=P))
        return b

    # load order: K-proj operands first so PE work starts ASAP
    wk = ld_w("wk", C, D)
    bk = ld_b("bk", C)
    wv = ld_w("wv", C, D)
    wq = ld_w("wq", C, D)
    wo = ld_w("wo", C, D)
    w1d = t["w1"].rearrange("(c p) o -> p c o", p=P)
    w2d = t["w2"].rearrange("(c p) o -> p c o", p=P)

    bq = ld_b("bq", C)
    bo = ld_b("bo", C)
    b2 = ld_b("b2", C)
    b1 = ld_b("b1", CF)
    g1 = ld_b("g1", C)
    be1 = ld_b("be1", C)
    g2 = ld_b("g2", C)
    be2 = ld_b("be2", C)

    bvb = wp.tile([P, D], F32)
    nc.gpsimd.dma_start(bvb, t["bv"][None, :].to_broadcast((P, D)))

    ones = wp.tile([P, 1], F32)
    nc.vector.memset(ones, 1.0)
    epst = wp.tile([1, 1], F32)
    nc.vector.memset(epst, LN_EPS)

    # persistent activations (full key sequence)
    kt = ap_.tile([P, C, TK], BF16)  # K.T
    va = ap_.tile([P, KC, H, DH + 1], BF16)  # V token-major, per head + ones col

    nc.vector.memset(va[:, :, :, DH : DH + 1], 1.0)

    xq32d = t["xq32"].rearrange("(c p) q -> p c q", p=P)
    xkb = t["xkb"].rearrange("(c p) q -> p c q", p=P)
    xvb = t["xvb"].rearrange("(c p) q -> p c q", p=P)
    out_d = t["out"].rearrange("(c p) q -> p c q", p=P)

    # ---------------- phase A: K/V projections (full sequence) ----------------
    for tk in range(TK // NT):
        ts_ = slice(tk * NT, (tk + 1) * NT)
        kr = []
        for ki in range(C):
            r = stream.tile([P, NT], BF16, tag="xr", name=f"kr_{tk}_{ki}")
            nc.sync.dma_start(r, xkb[:, ki, ts_])
            kr.append(r)
        for co in range(C):
            ps = psA.tile([P, NT], F32, tag="ps", name=f"kps_{tk}_{co}")
            for ki in range(C):
                nc.tensor.matmul(
                    ps,
                    wk[:, ki, co * P : (co + 1) * P],
                    kr[ki],
                    start=(ki == 0),
                    stop=(ki == C - 1),
                )
            nc.vector.tensor_scalar(
                out=kt[:, co, ts_], in0=ps, scalar1=bk[:, co : co + 1],
                scalar2=None, op0=OP.add,
            )

    # V projection, token-major out: V = Xv @ Wv  (lhsT = Xv.T chunk)
    for tm in range(KC):
        vl = []
        for ki in range(C):
            r = stream.tile([P, P], BF16, tag="vl", name=f"vl_{tm}_{ki}")
            nc.sync.dma_start(r, xvb[:, ki, tm * P : (tm + 1) * P])
            vl.append(r)
        ps = psA.tile([P, NT], F32, tag="ps", name=f"vps_{tm}")
        for ki in range(C):
            nc.tensor.matmul(ps, vl[ki], wv[:, ki, :], start=(ki == 0), stop=(ki == C - 1))
        for h in range(H):
            nc.vector.tensor_tensor(
                out=va[:, tm, h, 0:DH],
                in0=ps[:, h * DH : (h + 1) * DH],
                in1=bvb[:, h * DH : (h + 1) * DH],
                op=OP.add,
            )

    # ---------------- per query-token-tile: Qproj, attention, tail ----------------
    def layernorm(resid, g, be, out_write):
        """resid: [P, C, NT] f32 tile. out_write(co, t2_f32_tile, be_col)."""
        pm = psA.tile([P, NT], F32, tag="ps", name="ln_pm")
        for co in range(C):
            nc.tensor.matmul(pm[0:1, :], ones, resid[:, co, :], start=(co == 0), stop=(co == C - 1))
        sq = []
        for co in range(C):
            s = chunk.tile([P, NT], F32, tag="sqc", name=f"sq_{co}")
            nc.vector.tensor_mul(s, resid[:, co, :], resid[:, co, :])
            sq.append(s)
        pq = psA.tile([P, NT], F32, tag="ps", name="ln_pq")
        for co in range(C):
            nc.tensor.matmul(pq[0:1, :], ones, sq[co], start=(co == 0), stop=(co == C - 1))
        mean = small.tile([1, NT], F32, tag="sm", name="ln_mean")
        nc.vector.tensor_scalar_mul(mean, pm[0:1, :], 1.0 / D)
        msq = small.tile([1, NT], F32, tag="sm", name="ln_msq")
        nc.vector.tensor_scalar_mul(msq, pq[0:1, :], 1.0 / D)
        m2 = small.tile([1, NT], F32, tag="sm", name="ln_m2")
        nc.vector.tensor_mul(m2, mean, mean)
        var = small.tile([1, NT], F32, tag="sm", name="ln_var")
        nc.vector.tensor_tensor(out=var, in0=msq, in1=m2, op=OP.subtract)
        # rstd = exp(-0.5 * ln(var + eps)) -- stays in the Exp/Ln ACT table set
        lnv = small.tile([1, NT], F32, tag="sm", name="ln_lnv")
        nc.scalar.activation(lnv, var, AF.Ln, bias=epst)
        rstd = small.tile([1, NT], F32, tag="sm", name="ln_rstd")
        nc.scalar.activation(rstd, lnv, AF.Exp, scale=-0.5)
        meanb = chunk.tile([P, NT], F32, tag="bc", name="ln_meanb")
        nc.gpsimd.partition_broadcast(meanb, mean)
        rstdb = chunk.tile([P, NT], F32, tag="bc", name="ln_rstdb")
        nc.gpsimd.partition_broadcast(rstdb, rstd)
        for co in range(C):
            tt = chunk.tile([P, NT], F32, tag="tt", name=f"ln_tt_{co}")
            nc.vector.tensor_tensor(out=tt, in0=resid[:, co, :], in1=meanb, op=OP.subtract)
            t2 = chunk.tile([P, NT], F32, tag="t2", name=f"ln_t2_{co}")
            nc.vector.scalar_tensor_tensor(
                out=t2, in0=tt, scalar=g[:, co : co + 1], in1=rstdb, op0=OP.mult, op1=OP.mult
            )
            out_write(co, t2, be[:, co : co + 1])

    for tq in range(NQ):
        ts_ = slice(tq * NT, (tq + 1) * NT)

        # Q projection for this token tile
        xq32 = stage.tile([P, C, NT], F32, tag="xq32", name=f"xq32_{tq}")
        nc.sync.dma_start(xq32, xq32d[:, :, ts_])
        qt = ptq.tile([P, C, NT], BF16, tag="qt", name=f"qt_{tq}")
        qr = []
        for ki in range(C):
            r = stream.tile([P, NT], BF16, tag="xr", name=f"qr_{tq}_{ki}")
            nc.vector.tensor_copy(out=r, in_=xq32[:, ki, :])
            qr.append(r)
        for co in range(C):
            ps = psA.tile([P, NT], F32, tag="ps", name=f"qps_{tq}_{co}")
            for ki in range(C):
                nc.tensor.matmul(
                    ps,
                    wq[:, ki, co * P : (co + 1) * P],
                    qr[ki],
                    start=(ki == 0),
                    stop=(ki == C - 1),
                )
            nc.vector.tensor_scalar(
                out=qt[:, co, :], in0=ps, scalar1=bq[:, co : co + 1],
                scalar2=None, op0=OP.add,
            )

        # ---- attention ----
        ctx = ptq.tile([P, C, NT], BF16, tag="ctx", name=f"ctx_{tq}")
        for hp in range(H // 2):  # head pairs sharing a 128-partition chunk
            pc = [
                psC.tile([P, NT], F32, tag="pc", name=f"pc_{tq}_{hp}_{j}")
                for j in range(2)
            ]
            # software-pipelined: emit scores(kc)+exp(kc) BEFORE ctx(kc-1) so
            # the in-order PE stream always has independent work while the
            # exp for the current chunk is still on ScalarE.
            e2s = [None] * KC
            for kc in range(KC + 1):
                if kc < KC:
                    ksl = slice(kc * P, (kc + 1) * P)
                    # both heads' scores into one 2-bank PSUM tile -> one exp
                    ps2 = psS.tile(
                        [P, 2, NT], F32, tag="ps2", name=f"sps_{tq}_{hp}_{kc}"
                    )
                    e2 = epool.tile(
                        [P, 2, NT], BF16, tag="e", name=f"e_{tq}_{hp}_{kc}"
                    )
                    for j in range(2):  # head 2*hp + j at partition offset 64*j
                        rows = slice(j * DH, (j + 1) * DH)
                        # scores.T chunk = K_h @ Q_h.T
                        nc.tensor.matmul(
                            ps2[:, j, :], kt[rows, hp, ksl], qt[rows, hp, :],
                            start=True, stop=True,
                        )
                    nc.scalar.activation(e2, ps2, AF.Exp, scale=SCALE)
                    e2s[kc] = e2
                if kc >= 1:
                    for j in range(2):
                        # ctx.T (+ sumexp row 64) accumulate:
                        # lhsT = [V_h | 1] token chunk, rhs = E.T chunk
                        nc.tensor.matmul(
                            pc[j][0 : DH + 1, :],
                            va[:, kc - 1, 2 * hp + j, :],
                            e2s[kc - 1][:, j, :],
                            start=(kc - 1 == 0),
                            stop=(kc - 1 == KC - 1),
                        )
            for j in range(2):
                # copy out of PSUM promptly so the accumulator bank frees for
                # the next head pair; normalize from SBUF off the critical path
                # fast copies release the PSUM accumulator; the slow
                # reciprocal then runs off the critical path from SBUF
                se = small.tile([1, NT], F32, tag="sm", name=f"se_{tq}_{hp}_{j}")
                nc.vector.tensor_copy(out=se, in_=pc[j][DH : DH + 1, :])
                cf = chunk.tile([DH, NT], F32, tag="cf", name=f"cf_{tq}_{hp}_{j}")
                nc.vector.tensor_copy(out=cf, in_=pc[j][0:DH, :])
                rc = small.tile([1, NT], F32, tag="sm", name=f"rc_{tq}_{hp}_{j}")
                nc.vector.reciprocal(rc, se)
                db = chunk.tile([DH, NT], F32, tag="db", name=f"db_{tq}_{hp}_{j}")
                nc.gpsimd.partition_broadcast(db, rc)
                nc.vector.tensor_tensor(
                    out=ctx[j * DH : (j + 1) * DH, hp, :],
                    in0=cf,
                    in1=db,
                    op=OP.mult,
                )

        # ---- O projection + residual (query + attn_out) ----
        resid = stage.tile([P, C, NT], F32, tag="resid", name=f"resid_{tq}")
        for co in range(C):
            ps = psA.tile([P, NT], F32, tag="ps", name=f"ops_{tq}_{co}")
            for ki in range(C):
                nc.tensor.matmul(
                    ps,
                    wo[:, ki, co * P : (co + 1) * P],
                    ctx[:, ki, :],
                    start=(ki == 0),
                    stop=(ki == C - 1),
                )
            nc.vector.scalar_tensor_tensor(
                out=resid[:, co, :],
                in0=ps,
                scalar=bo[:, co : co + 1],
                in1=xq32[:, co, :],
                op0=OP.add,
                op1=OP.add,
            )

        # ---- LN1 -> ln1f (f32, kept for resid2) + ln1b (bf16, FFN rhs) ----
        ln1f = stage.tile([P, C, NT], F32, tag="ln1f", name=f"ln1f_{tq}")
        ln1b = ptq.tile([P, C, NT], BF16, tag="ln1b", name=f"ln1b_{tq}")

        def write_ln1(co, t2, bec, ln1f=ln1f, ln1b=ln1b):
            nc.vector.tensor_scalar(
                out=ln1f[:, co, :], in0=t2, scalar1=bec, scalar2=None, op0=OP.add
            )
            nc.vector.tensor_copy(out=ln1b[:, co, :], in_=ln1f[:, co, :])

        layernorm(resid, g1, be1, write_ln1)

        # ---- FFN1 + exact gelu ----
        hb = hpool.tile([P, CF, NT], BF16, tag="h", name=f"h_{tq}")
        for fo in range(CF):
            w1t = wstream.tile([P, C, P], BF16, tag="w1t", name=f"w1t_{tq}_{fo}")
            nc.sync.dma_start(w1t, w1d[:, :, fo * P : (fo + 1) * P])
            ps = psA.tile([P, NT], F32, tag="ps", name=f"fps_{tq}_{fo}")
            for ki in range(C):
                nc.tensor.matmul(
                    ps,
                    w1t[:, ki, :],
                    ln1b[:, ki, :],
                    start=(ki == 0),
                    stop=(ki == C - 1),
                )
            nc.scalar.activation(hb[:, fo, :], ps, AF.Gelu, bias=b1[:, fo : fo + 1])

        # ---- FFN2 + residual2 ----
        resid2 = stage.tile([P, C, NT], F32, tag="resid2", name=f"resid2_{tq}")
        for co in range(C):
            w2t = wstream.tile([P, CF, P], BF16, tag="w2t", name=f"w2t_{tq}_{co}")
            nc.sync.dma_start(w2t, w2d[:, :, co * P : (co + 1) * P])
            ps = psA.tile([P, NT], F32, tag="ps", name=f"gps_{tq}_{co}")
            for ki in range(CF):
                nc.tensor.matmul(
                    ps,
                    w2t[:, ki, :],
                    hb[:, ki, :],
                    start=(ki == 0),
                    stop=(ki == CF - 1),
                )
            nc.vector.scalar_tensor_tensor(
                out=resid2[:, co, :],
                in0=ps,
                scalar=b2[:, co : co + 1],
                in1=ln1f[:, co, :],
                op0=OP.add,
                op1=OP.add,
            )

        # ---- LN2 -> final output chunks -> DRAM ----
        def write_out(co, t2, bec, ts_=ts_):
            oc = chunk.tile([P, NT], F32, tag="oc", name=f"oc_{tq}_{co}")
            nc.vector.tensor_scalar(out=oc, in0=t2, scalar1=bec, scalar2=None, op0=OP.add)
            nc.sync.dma_start(out_d[:, co, ts_], oc)

        layernorm(resid2, g2, be2, write_out)


@functools.lru_cache(maxsize=1)
def build():
    from contextlib import ExitStack

    nc = bacc.Bacc("TRN2", target_bir_lowering=False, debug=False, num_devices=NCORES)
    t = {}

    def din(name, shape, dt):
        t[name] = nc.dram_tensor(name, list(shape), dt, kind="ExternalInput").ap()

    din("xq32", (D, TQ), F32)
    din("xkb", (D, TK), BF16)
    din("xvb", (D, TK), BF16)
    for w in ("wq", "wk", "wv", "wo"):
        din(w, (D, D), BF16)
    din("w1", (D, FF), BF16)
    din("w2", (FF, D), BF16)
    for b in ("bq", "bk", "bv", "bo", "b2", "g1", "be1", "g2", "be2"):
        din(b, (D,), F32)
    din("b1", (FF,), F32)
    t["out"] = nc.dram_tensor("out", [D, TQ], F32, kind="ExternalOutput").ap()

    with tile.TileContext(nc) as tc:
        with ExitStack() as es:
            _emit(nc, t, es, tc)
    nc.compile()
    return nc


def make_in_maps(query, key, value, Wq, bq, Wk, bk, Wv, bv, Wo, bo,
                 g1, be1, g2, be2, W1, b1, W2, b2):
    bf = ml_dtypes.bfloat16
    shared = {
        "wq": np.ascontiguousarray(Wq.astype(bf)),
        "wk": np.ascontiguousarray(Wk.astype(bf)),
        "wv": np.ascontiguousarray(Wv.astype(bf)),
        "wo": np.ascontiguousarray(Wo.astype(bf)),
        "w1": np.ascontiguousarray(W1.astype(bf)),
        "w2": np.ascontiguousarray(W2.astype(bf)),
        "bq": np.asarray(bq, np.float32), "bk": np.asarray(bk, np.float32),
        "bv": np.asarray(bv, np.float32), "bo": np.asarray(bo, np.float32),
        "b1": np.asarray(b1, np.float32), "b2": np.asarray(b2, np.float32),
        "g1": np.asarray(g1, np.float32), "be1": np.asarray(be1, np.float32),
        "g2": np.asarray(g2, np.float32), "be2": np.asarray(be2, np.float32),
    }
    in_maps = []
    for core in range(NCORES):
        b, half = divmod(core, 2)
        qsl = slice(half * TQ, (half + 1) * TQ)
        xq_t = np.ascontiguousarray(np.asarray(query[b, qsl], np.float32).T)
        xk_t = np.ascontiguousarray(np.asarray(key[b], np.float32).T.astype(bf))
        xv_t = np.ascontiguousarray(np.asarray(value[b], np.float32).T.astype(bf))
        in_maps.append({"xq32": xq_t, "xkb": xk_t, "xvb": xv_t, **shared})
    return in_maps


def kernel(**inputs):
    nc = build()
    in_maps = make_in_maps(**inputs)
    res = run_bass_kernel_spmd(nc, in_maps, list(range(NCORES)))
    out = np.empty((B, S, D), np.float32)
    for core in range(NCORES):
        b, half = divmod(core, 2)
        out[b, half * TQ : (half + 1) * TQ] = res.results[core]["out"].T
    return out


if __name__ == "__main__":
    import reference

    inputs = {k: np.asarray(v) for k, v in reference.setup_inputs().items()}
    got = kernel(**inputs)
    exp = np.asarray(reference.reference(**inputs))
    err = np.abs(got - exp).max() / np.abs(exp).max()
    print("rel err:", err)


# revision 23
# speedup vs baseline: 1.0408x; 1.0408x over previous
"""Trainium2 Bass kernel for a cross-modal transformer block (attention + FFN).

Contract: kernel(**inputs) takes the FULL unsharded inputs (numpy, fp32) and
returns the FULL output [4, 2048, 512] fp32.

Sharding: 8 cores = data-parallel over batch (4) x query-sequence halves (2).
Each core computes K/V projections for its batch's full 2048-token sequence
(cheap duplication) so attention needs no collectives.

Device layout: everything feature-major ([features on partitions, tokens on
free]); the host pre-transposes and pre-casts inputs so the device does zero
transposes.
"""

import functools
import sys

import numpy as np

sys.path.insert(0, "/opt/trn_rl_repo")

import ml_dtypes  # noqa: E402

import concourse.bass as bass  # noqa: E402
import concourse.tile as tile  # noqa: E402
from concourse import bacc, mybir  # noqa: E402
from concourse.bass_utils import run_bass_kernel_spmd  # noqa: E402

BF16 = mybir.dt.bfloat16
F32 = mybir.dt.float32
AF = mybir.ActivationFunctionType
OP = mybir.AluOpType

B, S, D = 4, 2048, 512
H, DH = 8, 64
FF = 2048
P = 128
C = D // P  # 4 feature chunks
CF = FF // P  # 16 ffn chunks
TQ = S // 2  # 1024 query tokens per core
TK = S  # full key sequence per core
KC = TK // P  # 16 key chunks
NT = 512  # token tile (matmul free dim)
NQ = TQ // NT  # 2 query-token tiles
SCALE = 1.0 / np.sqrt(DH)  # 0.125
LN_EPS = 1e-5
NCORES = 8


def _emit(nc, t, es, tc):
    """Emit the per-core program. t: dict name -> DRAM AP."""
    # ---------------- pools ----------------
    pw = es.enter_context(tc.tile_pool(name="projw", bufs=1))
    wp = es.enter_context(tc.tile_pool(name="w", bufs=1))
    ap_ = es.enter_context(tc.tile_pool(name="acts", bufs=1))
    ptq = es.enter_context(tc.tile_pool(name="ptq", bufs=2))
    psA = es.enter_context(tc.tile_pool(name="psA", bufs=2, space="PSUM"))
    psS = es.enter_context(tc.tile_pool(name="psS", bufs=2, space="PSUM"))
    psC = es.enter_context(tc.tile_pool(name="psC", bufs=2, space="PSUM"))
    stream = es.enter_context(tc.tile_pool(name="stream", bufs=5))
    wstream = es.enter_context(tc.tile_pool(name="wstream", bufs=2))
    stage = es.enter_context(tc.tile_pool(name="stage", bufs=2))
    stage1 = es.enter_context(tc.tile_pool(name="stage1", bufs=1))
    chunk = es.enter_context(tc.tile_pool(name="chunk", bufs=2))
    chunk3 = es.enter_context(tc.tile_pool(name="chunk3", bufs=3))
    small = es.enter_context(tc.tile_pool(name="small", bufs=4))
    epool = es.enter_context(tc.tile_pool(name="e", bufs=3))
    hpool = es.enter_context(tc.tile_pool(name="h", bufs=1))

    def ld_w(pool, name, kchunks, n):
        w = pool.tile([P, kchunks, n], BF16, name=name + "_sb")
        nc.sync.dma_start(w, t[name].rearrange("(c p) o -> p c o", p=P))
        return w

    def ld_b(name, kchunks):
        b = wp.tile([P, kchunks], F32, name=name + "_sb")
        nc.sync.dma_start(b, t[name].rearrange("(c p) -> p c", p=P))
        return b

    wk = ld_w(pw, "wk", C, D)
    bk = ld_b("bk", C)
    wv = ld_w(pw, "wv", C, D)
    wq = ld_w(pw, "wq", C, D)
    bvb = pw.tile([P, D], F32)
    nc.gpsimd.dma_start(bvb, t["bv"][None, :].to_broadcast((P, D)))

    wo = ld_w(wp, "wo", C, D)
    w1d = t["w1"].rearrange("(c p) o -> p c o", p=P)
    w2d = t["w2"].rearrange("(c p) o -> p c o", p=P)

    bq = ld_b("bq", C)
    bo = ld_b("bo", C)
    b2 = ld_b("b2", C)
    b1 = ld_b("b1", CF)
    g1 = ld_b("g1", C)
    be1 = ld_b("be1", C)
    g2 = ld_b("g2", C)
    be2 = ld_b("be2", C)

    ones = wp.tile([P, 1], F32)
    nc.vector.memset(ones, 1.0)
    epst = wp.tile([1, 1], F32)
    nc.vector.memset(epst, LN_EPS)

    # persistent activations (full key sequence)
    kt = ap_.tile([P, C, TK], BF16)  # K.T
    va = ap_.tile([P, KC, H, DH + 1], BF16)  # V token-major, per head + ones col

    nc.vector.memset(va[:, :, :, DH : DH + 1], 1.0)

    xq32d = t["xq32"].rearrange("(c p) q -> p c q", p=P)
    xqbd = t["xqb"].rearrange("(c p) q -> p c q", p=P)
    xkb = t["xkb"].rearrange("(c p) q -> p c q", p=P)
    xvb = t["xvb"].rearrange("(c p) q -> p c q", p=P)
    out_d = t["out"].rearrange("(c p) q -> p c q", p=P)

    # ---------------- phase A: K/V/Q projections ----------------
    for tk in range(TK // NT):
        ts_ = slice(tk * NT, (tk + 1) * NT)
        kr = []
        for ki in range(C):
            r = stream.tile([P, NT], BF16, tag="xr", name=f"kr_{tk}_{ki}")
            nc.sync.dma_start(r, xkb[:, ki, ts_])
            kr.append(r)
        for co in range(C):
            ps = psA.tile([P, NT], F32, tag="ps", name=f"kps_{tk}_{co}")
            for ki in range(C):
                nc.tensor.matmul(
                    ps,
                    wk[:, ki, co * P : (co + 1) * P],
                    kr[ki],
                    start=(ki == 0),
                    stop=(ki == C - 1),
                )
            nc.vector.tensor_scalar(
                out=kt[:, co, ts_], in0=ps, scalar1=bk[:, co : co + 1],
                scalar2=None, op0=OP.add,
            )

    # V projection, token-major out: V = Xv @ Wv  (lhsT = Xv.T chunk)
    for tm in range(KC):
        vl = []
        for ki in range(C):
            r = stream.tile([P, P], BF16, tag="vl", name=f"vl_{tm}_{ki}")
            nc.sync.dma_start(r, xvb[:, ki, tm * P : (tm + 1) * P])
            vl.append(r)
        ps = psA.tile([P, NT], F32, tag="ps", name=f"vps_{tm}")
        for ki in range(C):
            nc.tensor.matmul(ps, vl[ki], wv[:, ki, :], start=(ki == 0), stop=(ki == C - 1))
        for h in range(H):
            nc.vector.tensor_tensor(
                out=va[:, tm, h, 0:DH],
                in0=ps[:, h * DH : (h + 1) * DH],
                in1=bvb[:, h * DH : (h + 1) * DH],
                op=OP.add,
            )

    # Q projections for both token tiles (so attention for either tile is
    # never blocked on projection work)
    qts = []
    for tq in range(NQ):
        ts_ = slice(tq * NT, (tq + 1) * NT)
        qt = ptq.tile([P, C, NT], BF16, tag="qt", name=f"qt_{tq}")
        qr = []
        for ki in range(C):
            r = stream.tile([P, NT], BF16, tag="xr", name=f"qr_{tq}_{ki}")
            nc.sync.dma_start(r, xqbd[:, ki, ts_])
            qr.append(r)
        for co in range(C):
            ps = psA.tile([P, NT], F32, tag="ps", name=f"qps_{tq}_{co}")
            for ki in range(C):
                nc.tensor.matmul(
                    ps,
                    wq[:, ki, co * P : (co + 1) * P],
                    qr[ki],
                    start=(ki == 0),
                    stop=(ki == C - 1),
                )
            nc.vector.tensor_scalar(
                out=qt[:, co, :], in0=ps, scalar1=bq[:, co : co + 1],
                scalar2=None, op0=OP.add,
            )
        qts.append(qt)

    # ---------------- phase B: attention (both token tiles) ----------------
    ctxs = []
    for tq in range(NQ):
        qt = qts[tq]
        ctx = ptq.tile([P, C, NT], BF16, tag="ctx", name=f"ctx_{tq}")
        for hp in range(H // 2):  # head pairs sharing a 128-partition chunk
            pc = [
                psC.tile([P, NT], F32, tag="pc", name=f"pc_{tq}_{hp}_{j}")
                for j in range(2)
            ]
            # software-pipelined: scores(kc)+exp(kc) emitted before ctx(kc-1)
            e2s = [None] * KC
            for kc in range(KC + 1):
                if kc < KC:
                    ksl = slice(kc * P, (kc + 1) * P)
                    ps2 = psS.tile(
                        [P, 2, NT], F32, tag="ps2", name=f"sps_{tq}_{hp}_{kc}"
                    )
                    e2 = epool.tile(
                        [P, 2, NT], BF16, tag="e", name=f"e_{tq}_{hp}_{kc}"
                    )
                    for j in range(2):  # head 2*hp + j at partition offset 64*j
                        rows = slice(j * DH, (j + 1) * DH)
                        # scores.T chunk = K_h @ Q_h.T
                        nc.tensor.matmul(
                            ps2[:, j, :], kt[rows, hp, ksl], qt[rows, hp, :],
                            start=True, stop=True,
                        )
                    nc.scalar.activation(e2, ps2, AF.Exp, scale=SCALE)
                    e2s[kc] = e2
                if kc >= 1:
                    for j in range(2):
                        # ctx.T (+ sumexp row 64): lhsT = [V_h | 1], rhs = E.T
                        nc.tensor.matmul(
                            pc[j][0 : DH + 1, :],
                            va[:, kc - 1, 2 * hp + j, :],
                            e2s[kc - 1][:, j, :],
                            start=(kc - 1 == 0),
                            stop=(kc - 1 == KC - 1),
                        )
            for j in range(2):
                # fast copies release the PSUM accumulator; approx reciprocal
                # (~4e-6 rel err, plenty for a softmax denominator) keeps the
                # DVE FIFO clear
                se = small.tile([1, NT], F32, tag="sm", name=f"se_{tq}_{hp}_{j}")
                nc.vector.tensor_copy(out=se, in_=pc[j][DH : DH + 1, :])
                cf = chunk.tile([DH, NT], F32, tag="cf", name=f"cf_{tq}_{hp}_{j}")
                nc.vector.tensor_copy(out=cf, in_=pc[j][0:DH, :])
                rc = small.tile([1, NT], F32, tag="sm", name=f"rc_{tq}_{hp}_{j}")
                nc.vector.reciprocal_approx_fast(out=rc, in_=se)
                db = chunk.tile([DH, NT], F32, tag="db", name=f"db_{tq}_{hp}_{j}")
                nc.gpsimd.partition_broadcast(db, rc)
                nc.vector.tensor_tensor(
                    out=ctx[j * DH : (j + 1) * DH, hp, :],
                    in0=cf,
                    in1=db,
                    op=OP.mult,
                )
        ctxs.append(ctx)

    # ---------------- phase C: tails, interleaved across token tiles ------
    def layernorm(resid, g, be, out_write, tag):
        """resid: [P, C, NT] f32 tile. out_write(co, t2_f32_tile, be_col)."""
        pm = psA.tile([P, NT], F32, tag="ps", name=f"pm_{tag}")
        for co in range(C):
            nc.tensor.matmul(pm[0:1, :], ones, resid[:, co, :], start=(co == 0), stop=(co == C - 1))
        sq = []
        for co in range(C):
            s = chunk.tile([P, NT], F32, tag="sqc", name=f"sq_{tag}_{co}")
            nc.vector.tensor_mul(s, resid[:, co, :], resid[:, co, :])
            sq.append(s)
        pq = psA.tile([P, NT], F32, tag="ps", name=f"pq_{tag}")
        for co in range(C):
            nc.tensor.matmul(pq[0:1, :], ones, sq[co], start=(co == 0), stop=(co == C - 1))
        mean = small.tile([1, NT], F32, tag="sm", name=f"mean_{tag}")
        nc.vector.tensor_scalar_mul(mean, pm[0:1, :], 1.0 / D)
        msq = small.tile([1, NT], F32, tag="sm", name=f"msq_{tag}")
        nc.vector.tensor_scalar_mul(msq, pq[0:1, :], 1.0 / D)
        m2 = small.tile([1, NT], F32, tag="sm", name=f"m2_{tag}")
        nc.vector.tensor_mul(m2, mean, mean)
        var = small.tile([1, NT], F32, tag="sm", name=f"var_{tag}")
        nc.vector.tensor_tensor(out=var, in0=msq, in1=m2, op=OP.subtract)
        # rstd = exp(-0.5 * ln(var + eps)) -- stays in the Exp/Ln ACT table set
        lnv = small.tile([1, NT], F32, tag="sm", name=f"lnv_{tag}")
        nc.scalar.activation(lnv, var, AF.Ln, bias=epst)
        rstd = small.tile([1, NT], F32, tag="sm", name=f"rstd_{tag}")
        nc.scalar.activation(rstd, lnv, AF.Exp, scale=-0.5)
        meanb = chunk.tile([P, NT], F32, tag="bc", name=f"meanb_{tag}")
        nc.gpsimd.partition_broadcast(meanb, mean)
        rstdb = chunk.tile([P, NT], F32, tag="bc", name=f"rstdb_{tag}")
        nc.gpsimd.partition_broadcast(rstdb, rstd)
        for co in range(C):
            tt = chunk3.tile([P, NT], F32, tag="tmp", name=f"tt_{tag}_{co}")
            nc.vector.tensor_tensor(out=tt, in0=resid[:, co, :], in1=meanb, op=OP.subtract)
            t2 = chunk3.tile([P, NT], F32, tag="tmp", name=f"t2_{tag}_{co}")
            nc.vector.scalar_tensor_tensor(
                out=t2, in0=tt, scalar=g[:, co : co + 1], in1=rstdb, op0=OP.mult, op1=OP.mult
            )
            out_write(co, t2, be[:, co : co + 1])

    # O projection + residual (query + attn_out)
    resids = []
    for tq in range(NQ):
        ts_ = slice(tq * NT, (tq + 1) * NT)
        ctx = ctxs[tq]
        resid = stage.tile([P, C, NT], F32, tag="resid", name=f"resid_{tq}")
        for co in range(C):
            xqc = chunk.tile([P, NT], F32, tag="xqc", name=f"xqc_{tq}_{co}")
            nc.sync.dma_start(xqc, xq32d[:, co, ts_])
            ps = psA.tile([P, NT], F32, tag="ps", name=f"ops_{tq}_{co}")
            for ki in range(C):
                nc.tensor.matmul(
                    ps,
                    wo[:, ki, co * P : (co + 1) * P],
                    ctx[:, ki, :],
                    start=(ki == 0),
                    stop=(ki == C - 1),
                )
            nc.vector.scalar_tensor_tensor(
                out=resid[:, co, :],
                in0=ps,
                scalar=bo[:, co : co + 1],
                in1=xqc,
                op0=OP.add,
                op1=OP.add,
            )
        resids.append(resid)

    # per-tq: LN1 -> FFN1 -> FFN2 -> LN2 (single-buffered ln1f/resid2/hb;
    # each tq's LN2 chain overlaps the next tq's FFN matmuls)
    for tq in range(NQ):
        ts_ = slice(tq * NT, (tq + 1) * NT)
        ln1f = stage1.tile([P, C, NT], F32, tag="ln1f", name=f"ln1f_{tq}")
        ln1b = stage1.tile([P, C, NT], BF16, tag="ln1b", name=f"ln1b_{tq}")

        def write_ln1(co, t2, bec, ln1f=ln1f, ln1b=ln1b):
            nc.vector.tensor_scalar(
                out=ln1f[:, co, :], in0=t2, scalar1=bec, scalar2=None, op0=OP.add
            )
            nc.vector.tensor_copy(out=ln1b[:, co, :], in_=ln1f[:, co, :])

        layernorm(resids[tq], g1, be1, write_ln1, f"l1_{tq}")

        hb = hpool.tile([P, CF, NT], BF16, tag="h", name=f"h_{tq}")
        for fo in range(CF):
            w1t = wstream.tile([P, C, P], BF16, tag="w1t", name=f"w1t_{tq}_{fo}")
            nc.sync.dma_start(w1t, w1d[:, :, fo * P : (fo + 1) * P])
            ps = psA.tile([P, NT], F32, tag="ps", name=f"fps_{tq}_{fo}")
            for ki in range(C):
                nc.tensor.matmul(
                    ps,
                    w1t[:, ki, :],
                    ln1b[:, ki, :],
                    start=(ki == 0),
                    stop=(ki == C - 1),
                )
            nc.scalar.activation(hb[:, fo, :], ps, AF.Gelu, bias=b1[:, fo : fo + 1])

        resid2 = stage1.tile([P, C, NT], F32, tag="resid2", name=f"resid2_{tq}")
        for co in range(C):
            w2t = wstream.tile([P, CF, P], BF16, tag="w2t", name=f"w2t_{tq}_{co}")
            nc.sync.dma_start(w2t, w2d[:, :, co * P : (co + 1) * P])
            ps = psA.tile([P, NT], F32, tag="ps", name=f"gps_{tq}_{co}")
            for ki in range(CF):
                nc.tensor.matmul(
                    ps,
                    w2t[:, ki, :],
                    hb[:, ki, :],
                    start=(ki == 0),
                    stop=(ki == CF - 1),
                )
            nc.vector.scalar_tensor_tensor(
                out=resid2[:, co, :],
                in0=ps,
                scalar=b2[:, co : co + 1],
                in1=ln1f[:, co, :],
                op0=OP.add,
                op1=OP.add,
            )

        def write_out(co, t2, bec, ts_=ts_, tq=tq):
            oc = chunk3.tile([P, NT], F32, tag="tmp", name=f"oc_{tq}_{co}")
            nc.vector.tensor_scalar(out=oc, in0=t2, scalar1=bec, scalar2=None, op0=OP.add)
            nc.sync.dma_start(out_d[:, co, ts_], oc)

        layernorm(resid2, g2, be2, write_out, f"l2_{tq}")


@functools.lru_cache(maxsize=1)
def build():
    from contextlib import ExitStack

    nc = bacc.Bacc("TRN2", target_bir_lowering=False, debug=False, num_devices=NCORES)
    t = {}

    def din(name, shape, dt):
        t[name] = nc.dram_tensor(name, list(shape), dt, kind="ExternalInput").ap()

    din("xq32", (D, TQ), F32)
    din("xqb", (D, TQ), BF16)
    din("xkb", (D, TK), BF16)
    din("xvb", (D, TK), BF16)
    for w in ("wq", "wk", "wv", "wo"):
        din(w, (D, D), BF16)
    din("w1", (D, FF), BF16)
    din("w2", (FF, D), BF16)
    for b in ("bq", "bk", "bv", "bo", "b2", "g1", "be1", "g2", "be2"):
        din(b, (D,), F32)
    din("b1", (FF,), F32)
    t["out"] = nc.dram_tensor("out", [D, TQ], F32, kind="ExternalOutput").ap()

    with tile.TileContext(nc) as tc:
        with ExitStack() as es:
            _emit(nc, t, es, tc)
    nc.compile()
    return nc


def make_in_maps(query, key, value, Wq, bq, Wk, bk, Wv, bv, Wo, bo,
                 g1, be1, g2, be2, W1, b1, W2, b2):
    bf = ml_dtypes.bfloat16
    shared = {
        "wq": np.ascontiguousarray(Wq.astype(bf)),
        "wk": np.ascontiguousarray(Wk.astype(bf)),
        "wv": np.ascontiguousarray(Wv.astype(bf)),
        "wo": np.ascontiguousarray(Wo.astype(bf)),
        "w1": np.ascontiguousarray(W1.astype(bf)),
        "w2": np.ascontiguousarray(W2.astype(bf)),
        "bq": np.asarray(bq, np.float32), "bk": np.asarray(bk, np.float32),
        "bv": np.asarray(bv, np.float32), "bo": np.asarray(bo, np.float32),
        "b1": np.asarray(b1, np.float32), "b2": np.asarray(b2, np.float32),
        "g1": np.asarray(g1, np.float32), "be1": np.asarray(be1, np.float32),
        "g2": np.asarray(g2, np.float32), "be2": np.asarray(be2, np.float32),
    }
    in_maps = []
    for core in range(NCORES):
        b, half = divmod(core, 2)
        qsl = slice(half * TQ, (half + 1) * TQ)
        xq_t = np.ascontiguousarray(np.asarray(query[b, qsl], np.float32).T)
        xk_t = np.ascontiguousarray(np.asarray(key[b], np.float32).T.astype(bf))
        xv_t = np.ascontiguousarray(np.asarray(value[b], np.float32).T.astype(bf))
        in_maps.append({
            "xq32": xq_t, "xqb": np.ascontiguousarray(xq_t.astype(bf)),
            "xkb": xk_t, "xvb": xv_t, **shared,
        })
    return in_maps


def kernel(**inputs):
    nc = build()
    in_maps = make_in_maps(**inputs)
    res = run_bass_kernel_spmd(nc, in_maps, list(range(NCORES)))
    out = np.empty((B, S, D), np.float32)
    for core in range(NCORES):
        b, half = divmod(core, 2)
        out[b, half * TQ : (half + 1) * TQ] = res.results[core]["out"].T
    return out


if __name__ == "__main__":
    import reference

    inputs = {k: np.asarray(v) for k, v in reference.setup_inputs().items()}
    got = kernel(**inputs)
    exp = np.asarray(reference.reference(**inputs))
    err = np.abs(got - exp).max() / np.abs(exp).max()
    print("rel err:", err)


# revision 24
# speedup vs baseline: 1.1685x; 1.1227x over previous
"""Trainium2 Bass kernel for a cross-modal transformer block (attention + FFN).

Contract: kernel(**inputs) takes the FULL unsharded inputs (numpy, fp32) and
returns the FULL output [4, 2048, 512] fp32.

Sharding: 8 cores = data-parallel over batch (4) x query-sequence halves (2).
Each core computes K/V projections for its batch's full 2048-token sequence
(cheap duplication) so attention needs no collectives.

Device layout: everything feature-major ([features on partitions, tokens on
free]); the host pre-transposes and pre-casts inputs so the device does zero
transposes.
"""

import functools
import sys

import numpy as np

sys.path.insert(0, "/opt/trn_rl_repo")

import ml_dtypes  # noqa: E402

import concourse.bass as bass  # noqa: E402
import concourse.tile as tile  # noqa: E402
from concourse import bacc, mybir  # noqa: E402
from concourse.bass_utils import run_bass_kernel_spmd  # noqa: E402

BF16 = mybir.dt.bfloat16
F32 = mybir.dt.float32
AF = mybir.ActivationFunctionType
OP = mybir.AluOpType

B, S, D = 4, 2048, 512
H, DH = 8, 64
FF = 2048
P = 128
C = D // P  # 4 feature chunks
CF = FF // P  # 16 ffn chunks
TQ = S // 2  # 1024 query tokens per core
TK = S  # full key sequence per core
KC = TK // P  # 16 key chunks
NT = 512  # token tile (matmul free dim)
NQ = TQ // NT  # 2 query-token tiles
SCALE = 1.0 / np.sqrt(DH)  # 0.125
LN_EPS = 1e-5
NCORES = 8


def _emit(nc, t, es, tc):
    """Emit the per-core program. t: dict name -> DRAM AP."""
    # ---------------- pools ----------------
    pw = es.enter_context(tc.tile_pool(name="projw", bufs=1))
    wp = es.enter_context(tc.tile_pool(name="w", bufs=1))
    ap_ = es.enter_context(tc.tile_pool(name="acts", bufs=1))
    ptq = es.enter_context(tc.tile_pool(name="ptq", bufs=2))
    psA = es.enter_context(tc.tile_pool(name="psA", bufs=2, space="PSUM"))
    psS = es.enter_context(tc.tile_pool(name="psS", bufs=2, space="PSUM"))
    psC = es.enter_context(tc.tile_pool(name="psC", bufs=2, space="PSUM"))
    stream = es.enter_context(tc.tile_pool(name="stream", bufs=6))
    stage = es.enter_context(tc.tile_pool(name="stage", bufs=2))
    stage1 = es.enter_context(tc.tile_pool(name="stage1", bufs=1))
    chunk = es.enter_context(tc.tile_pool(name="chunk", bufs=2))
    chunk3 = es.enter_context(tc.tile_pool(name="chunk3", bufs=3))
    small = es.enter_context(tc.tile_pool(name="small", bufs=6))
    epool = es.enter_context(tc.tile_pool(name="e", bufs=4))
    hpool = es.enter_context(tc.tile_pool(name="h", bufs=1))

    def ld_w(pool, name, kchunks, n):
        w = pool.tile([P, kchunks, n], BF16, name=name + "_sb")
        nc.sync.dma_start(w, t[name].rearrange("(c p) o -> p c o", p=P))
        return w

    def ld_b(name, kchunks):
        b = wp.tile([P, kchunks], F32, name=name + "_sb")
        nc.sync.dma_start(b, t[name].rearrange("(c p) -> p c", p=P))
        return b

    wk = ld_w(pw, "wk", C, D)
    bk = ld_b("bk", C)
    wv = ld_w(pw, "wv", C, D)
    wq = ld_w(pw, "wq", C, D)
    bvb = pw.tile([P, D], F32)
    nc.gpsimd.dma_start(bvb, t["bv"][None, :].to_broadcast((P, D)))

    wo = ld_w(wp, "wo", C, D)
    w1d = t["w1"].rearrange("(c p) o -> p c o", p=P)
    w2d = t["w2"].rearrange("(c p) o -> p c o", p=P)

    bq = ld_b("bq", C)
    bo = ld_b("bo", C)
    b2 = ld_b("b2", C)
    b1 = ld_b("b1", CF)
    g1 = ld_b("g1", C)
    be1 = ld_b("be1", C)
    g2 = ld_b("g2", C)
    be2 = ld_b("be2", C)

    ones = wp.tile([P, 1], F32)
    nc.vector.memset(ones, 1.0)
    epst = wp.tile([1, 1], F32)
    nc.vector.memset(epst, LN_EPS)

    # persistent activations (full key sequence); tags shared with the FFN
    # weights, which reuse these slots once attention is done
    kt = ap_.tile([P, C, TK], BF16, tag="big1", name="kt")  # K.T
    va = ap_.tile([P, KC, H, DH + 1], BF16, tag="big2", name="va")  # V + ones col

    nc.vector.memset(va[:, :, :, DH : DH + 1], 1.0)

    xq32d = t["xq32"].rearrange("(c p) q -> p c q", p=P)
    xqbd = t["xqb"].rearrange("(c p) q -> p c q", p=P)
    xkb = t["xkb"].rearrange("(c p) q -> p c q", p=P)
    xvb = t["xvb"].rearrange("(c p) q -> p c q", p=P)
    out_d = t["out"].rearrange("(c p) q -> p c q", p=P)

    # ---------------- phase A: K/V/Q projections ----------------
    # inputs are loaded as [P, 1024] tiles (2KB DMA lines)
    for half in range(2):
        hs = slice(half * 1024, (half + 1) * 1024)
        kr = []
        for ki in range(C):
            r = stream.tile([P, 1024], BF16, tag="xr", name=f"kr_{half}_{ki}")
            nc.sync.dma_start(r, xkb[:, ki, hs])
            kr.append(r)
        for tk in range(2):
            ts_ = slice(half * 1024 + tk * NT, half * 1024 + (tk + 1) * NT)
            tsl = slice(tk * NT, (tk + 1) * NT)
            for co in range(C):
                ps = psA.tile([P, NT], F32, tag="ps", name=f"kps_{half}_{tk}_{co}")
                for ki in range(C):
                    nc.tensor.matmul(
                        ps,
                        wk[:, ki, co * P : (co + 1) * P],
                        kr[ki][:, tsl],
                        start=(ki == 0),
                        stop=(ki == C - 1),
                    )
                nc.vector.tensor_scalar(
                    out=kt[:, co, ts_], in0=ps, scalar1=bk[:, co : co + 1],
                    scalar2=None, op0=OP.add,
                )

    # V projection, token-major out: V = Xv @ Wv  (lhsT = Xv.T chunk)
    for half in range(2):
        hs = slice(half * 1024, (half + 1) * 1024)
        vr = []
        for ki in range(C):
            r = stream.tile([P, 1024], BF16, tag="xr", name=f"vr_{half}_{ki}")
            nc.sync.dma_start(r, xvb[:, ki, hs])
            vr.append(r)
        for tm8 in range(8):
            tm = half * 8 + tm8
            msl = slice(tm8 * P, (tm8 + 1) * P)
            ps = psA.tile([P, NT], F32, tag="ps", name=f"vps_{tm}")
            for ki in range(C):
                nc.tensor.matmul(
                    ps, vr[ki][:, msl], wv[:, ki, :],
                    start=(ki == 0), stop=(ki == C - 1),
                )
            for h in range(H):
                nc.vector.tensor_tensor(
                    out=va[:, tm, h, 0:DH],
                    in0=ps[:, h * DH : (h + 1) * DH],
                    in1=bvb[:, h * DH : (h + 1) * DH],
                    op=OP.add,
                )

    # Q projections for both token tiles (so attention for either tile is
    # never blocked on projection work)
    qr = []
    for ki in range(C):
        r = stream.tile([P, TQ], BF16, tag="xr", name=f"qr_{ki}")
        nc.sync.dma_start(r, xqbd[:, ki, :])
        qr.append(r)
    qts = []
    for tq in range(NQ):
        tsl = slice(tq * NT, (tq + 1) * NT)
        qt = ptq.tile([P, C, NT], BF16, tag="qt", name=f"qt_{tq}")
        for co in range(C):
            ps = psA.tile([P, NT], F32, tag="ps", name=f"qps_{tq}_{co}")
            for ki in range(C):
                nc.tensor.matmul(
                    ps,
                    wq[:, ki, co * P : (co + 1) * P],
                    qr[ki][:, tsl],
                    start=(ki == 0),
                    stop=(ki == C - 1),
                )
            nc.vector.tensor_scalar(
                out=qt[:, co, :], in0=ps, scalar1=bq[:, co : co + 1],
                scalar2=None, op0=OP.add,
            )
        qts.append(qt)

    # ---------------- phase B: attention (both token tiles) ----------------
    ctxs = []
    for tq in range(NQ):
        qt = qts[tq]
        ctx = ptq.tile([P, C, NT], BF16, tag="ctx", name=f"ctx_{tq}")
        for hp in range(H // 2):  # head pairs sharing a 128-partition chunk
            pc = [
                psC.tile([P, NT], F32, tag="pc", name=f"pc_{tq}_{hp}_{j}")
                for j in range(2)
            ]
            # software-pipelined: scores(kc)+exp(kc) emitted before ctx(kc-1)
            e2s = [None] * KC
            for kc in range(KC + 1):
                if kc < KC:
                    ksl = slice(kc * P, (kc + 1) * P)
                    ps2 = psS.tile(
                        [P, 2, NT], F32, tag="ps2", name=f"sps_{tq}_{hp}_{kc}"
                    )
                    e2 = epool.tile(
                        [P, 2, NT], BF16, tag="e", name=f"e_{tq}_{hp}_{kc}"
                    )
                    for j in range(2):  # head 2*hp + j at partition offset 64*j
                        rows = slice(j * DH, (j + 1) * DH)
                        # scores.T chunk = K_h @ Q_h.T
                        nc.tensor.matmul(
                            ps2[:, j, :], kt[rows, hp, ksl], qt[rows, hp, :],
                            start=True, stop=True,
                        )
                    nc.scalar.activation(e2, ps2, AF.Exp, scale=SCALE)
                    e2s[kc] = e2
                if kc >= 1:
                    for j in range(2):
                        # ctx.T (+ sumexp row 64): lhsT = [V_h | 1], rhs = E.T
                        nc.tensor.matmul(
                            pc[j][0 : DH + 1, :],
                            va[:, kc - 1, 2 * hp + j, :],
                            e2s[kc - 1][:, j, :],
                            start=(kc - 1 == 0),
                            stop=(kc - 1 == KC - 1),
                        )
            for j in range(2):
                # fast copies release the PSUM accumulator; approx reciprocal
                # (~4e-6 rel err, plenty for a softmax denominator) keeps the
                # DVE FIFO clear
                se = small.tile([1, NT], F32, tag="sm", name=f"se_{tq}_{hp}_{j}")
                nc.vector.tensor_copy(out=se, in_=pc[j][DH : DH + 1, :])
                cf = chunk.tile([DH, NT], F32, tag="cf", name=f"cf_{tq}_{hp}_{j}")
                nc.vector.tensor_copy(out=cf, in_=pc[j][0:DH, :])
                rc = small.tile([1, NT], F32, tag="sm", name=f"rc_{tq}_{hp}_{j}")
                nc.vector.reciprocal_approx_fast(out=rc, in_=se)
                db = chunk.tile([DH, NT], F32, tag="db", name=f"db_{tq}_{hp}_{j}")
                nc.gpsimd.partition_broadcast(db, rc)
                nc.vector.tensor_tensor(
                    out=ctx[j * DH : (j + 1) * DH, hp, :],
                    in0=cf,
                    in1=db,
                    op=OP.mult,
                )
        ctxs.append(ctx)

    # FFN weights reuse the kt/va slots (attention is done with them);
    # 4 DMAs each so the loads spread across queues and overlap Oproj/LN1
    w1s = ap_.tile([P, C, FF], BF16, tag="big1", name="w1s")
    for ki in range(C):
        nc.sync.dma_start(w1s[:, ki, :], w1d[:, ki, :])
    w2s = ap_.tile([P, CF, D], BF16, tag="big2", name="w2s")
    for kq in range(4):
        nc.sync.dma_start(w2s[:, 4 * kq : 4 * kq + 4, :], w2d[:, 4 * kq : 4 * kq + 4, :])

    # ---------------- phase C: tails, interleaved across token tiles ------
    def layernorm(resid, g, be, out_write, tag):
        """resid: [P, C, NT] f32 tile. out_write(co, t2_f32_tile, be_col)."""
        pm = psA.tile([P, NT], F32, tag="ps", name=f"pm_{tag}")
        for co in range(C):
            nc.tensor.matmul(pm[0:1, :], ones, resid[:, co, :], start=(co == 0), stop=(co == C - 1))
        sq = []
        for co in range(C):
            s = chunk.tile([P, NT], F32, tag="sqc", name=f"sq_{tag}_{co}")
            nc.vector.tensor_mul(s, resid[:, co, :], resid[:, co, :])
            sq.append(s)
        pq = psA.tile([P, NT], F32, tag="ps", name=f"pq_{tag}")
        for co in range(C):
            nc.tensor.matmul(pq[0:1, :], ones, sq[co], start=(co == 0), stop=(co == C - 1))
        mean = small.tile([1, NT], F32, tag="sm", name=f"mean_{tag}")
        nc.vector.tensor_scalar_mul(mean, pm[0:1, :], 1.0 / D)
        msq = small.tile([1, NT], F32, tag="sm", name=f"msq_{tag}")
        nc.vector.tensor_scalar_mul(msq, pq[0:1, :], 1.0 / D)
        m2 = small.tile([1, NT], F32, tag="sm", name=f"m2_{tag}")
        nc.vector.tensor_mul(m2, mean, mean)
        var = small.tile([1, NT], F32, tag="sm", name=f"var_{tag}")
        nc.vector.tensor_tensor(out=var, in0=msq, in1=m2, op=OP.subtract)
        # rstd = exp(-0.5 * ln(var + eps)) -- stays in the Exp/Ln ACT table set
        lnv = small.tile([1, NT], F32, tag="sm", name=f"lnv_{tag}")
        nc.scalar.activation(lnv, var, AF.Ln, bias=epst)
        rstd = small.tile([1, NT], F32, tag="sm", name=f"rstd_{tag}")
        nc.scalar.activation(rstd, lnv, AF.Exp, scale=-0.5)
        meanb = chunk.tile([P, NT], F32, tag="bc", name=f"meanb_{tag}")
        nc.gpsimd.partition_broadcast(meanb, mean)
        rstdb = chunk.tile([P, NT], F32, tag="bc", name=f"rstdb_{tag}")
        nc.gpsimd.partition_broadcast(rstdb, rstd)
        for co in range(C):
            tt = chunk3.tile([P, NT], F32, tag="tmp", name=f"tt_{tag}_{co}")
            nc.vector.tensor_tensor(out=tt, in0=resid[:, co, :], in1=meanb, op=OP.subtract)
            t2 = chunk3.tile([P, NT], F32, tag="tmp", name=f"t2_{tag}_{co}")
            nc.vector.scalar_tensor_tensor(
                out=t2, in0=tt, scalar=g[:, co : co + 1], in1=rstdb, op0=OP.mult, op1=OP.mult
            )
            out_write(co, t2, be[:, co : co + 1])

    # O projection + residual (query + attn_out)
    resids = []
    for tq in range(NQ):
        ts_ = slice(tq * NT, (tq + 1) * NT)
        ctx = ctxs[tq]
        resid = stage.tile([P, C, NT], F32, tag="resid", name=f"resid_{tq}")
        for co in range(C):
            xqc = chunk.tile([P, NT], F32, tag="xqc", name=f"xqc_{tq}_{co}")
            nc.sync.dma_start(xqc, xq32d[:, co, ts_])
            ps = psA.tile([P, NT], F32, tag="ps", name=f"ops_{tq}_{co}")
            for ki in range(C):
                nc.tensor.matmul(
                    ps,
                    wo[:, ki, co * P : (co + 1) * P],
                    ctx[:, ki, :],
                    start=(ki == 0),
                    stop=(ki == C - 1),
                )
            nc.vector.scalar_tensor_tensor(
                out=resid[:, co, :],
                in0=ps,
                scalar=bo[:, co : co + 1],
                in1=xqc,
                op0=OP.add,
                op1=OP.add,
            )
        resids.append(resid)

    # per-tq: LN1 -> FFN1 -> FFN2 -> LN2 (single-buffered ln1f/resid2/hb;
    # each tq's LN2 chain overlaps the next tq's FFN matmuls)
    for tq in range(NQ):
        ts_ = slice(tq * NT, (tq + 1) * NT)
        ln1f = stage1.tile([P, C, NT], F32, tag="ln1f", name=f"ln1f_{tq}")
        ln1b = stage1.tile([P, C, NT], BF16, tag="ln1b", name=f"ln1b_{tq}")

        def write_ln1(co, t2, bec, ln1f=ln1f, ln1b=ln1b):
            nc.vector.tensor_scalar(
                out=ln1f[:, co, :], in0=t2, scalar1=bec, scalar2=None, op0=OP.add
            )
            nc.vector.tensor_copy(out=ln1b[:, co, :], in_=ln1f[:, co, :])

        layernorm(resids[tq], g1, be1, write_ln1, f"l1_{tq}")

        hb = hpool.tile([P, CF, NT], BF16, tag="h", name=f"h_{tq}")
        for fo in range(CF):
            ps = psA.tile([P, NT], F32, tag="ps", name=f"fps_{tq}_{fo}")
            for ki in range(C):
                nc.tensor.matmul(
                    ps,
                    w1s[:, ki, fo * P : (fo + 1) * P],
                    ln1b[:, ki, :],
                    start=(ki == 0),
                    stop=(ki == C - 1),
                )
            nc.scalar.activation(hb[:, fo, :], ps, AF.Gelu, bias=b1[:, fo : fo + 1])

        resid2 = stage1.tile([P, C, NT], F32, tag="resid2", name=f"resid2_{tq}")
        for co in range(C):
            ps = psA.tile([P, NT], F32, tag="ps", name=f"gps_{tq}_{co}")
            for ki in range(CF):
                nc.tensor.matmul(
                    ps,
                    w2s[:, ki, co * P : (co + 1) * P],
                    hb[:, ki, :],
                    start=(ki == 0),
                    stop=(ki == CF - 1),
                )
            nc.vector.scalar_tensor_tensor(
                out=resid2[:, co, :],
                in0=ps,
                scalar=b2[:, co : co + 1],
                in1=ln1f[:, co, :],
                op0=OP.add,
                op1=OP.add,
            )

        def write_out(co, t2, bec, ts_=ts_, tq=tq):
            oc = chunk3.tile([P, NT], F32, tag="tmp", name=f"oc_{tq}_{co}")
            nc.vector.tensor_scalar(out=oc, in0=t2, scalar1=bec, scalar2=None, op0=OP.add)
            nc.sync.dma_start(out_d[:, co, ts_], oc)

        layernorm(resid2, g2, be2, write_out, f"l2_{tq}")


@functools.lru_cache(maxsize=1)
def build():
    from contextlib import ExitStack

    nc = bacc.Bacc("TRN2", target_bir_lowering=False, debug=False, num_devices=NCORES)
    t = {}

    def din(name, shape, dt):
        t[name] = nc.dram_tensor(name, list(shape), dt, kind="ExternalInput").ap()

    din("xq32", (D, TQ), F32)
    din("xqb", (D, TQ), BF16)
    din("xkb", (D, TK), BF16)
    din("xvb", (D, TK), BF16)
    for w in ("wq", "wk", "wv", "wo"):
        din(w, (D, D), BF16)
    din("w1", (D, FF), BF16)
    din("w2", (FF, D), BF16)
    for b in ("bq", "bk", "bv", "bo", "b2", "g1", "be1", "g2", "be2"):
        din(b, (D,), F32)
    din("b1", (FF,), F32)
    t["out"] = nc.dram_tensor("out", [D, TQ], F32, kind="ExternalOutput").ap()

    with tile.TileContext(nc) as tc:
        with ExitStack() as es:
            _emit(nc, t, es, tc)
    nc.compile()
    return nc


def make_in_maps(query, key, value, Wq, bq, Wk, bk, Wv, bv, Wo, bo,
                 g1, be1, g2, be2, W1, b1, W2, b2):
    bf = ml_dtypes.bfloat16
    shared = {
        "wq": np.ascontiguousarray(Wq.astype(bf)),
        "wk": np.ascontiguousarray(Wk.astype(bf)),
        "wv": np.ascontiguousarray(Wv.astype(bf)),
        "wo": np.ascontiguousarray(Wo.astype(bf)),
        "w1": np.ascontiguousarray(W1.astype(bf)),
        "w2": np.ascontiguousarray(W2.astype(bf)),
        "bq": np.asarray(bq, np.float32), "bk": np.asarray(bk, np.float32),
        "bv": np.asarray(bv, np.float32), "bo": np.asarray(bo, np.float32),
        "b1": np.asarray(b1, np.float32), "b2": np.asarray(b2, np.float32),
        "g1": np.asarray(g1, np.float32), "be1": np.asarray(be1, np.float32),
        "g2": np.asarray(g2, np.float32), "be2": np.asarray(be2, np.float32),
    }
    in_maps = []
    for core in range(NCORES):
        b, half = divmod(core, 2)
        qsl = slice(half * TQ, (half + 1) * TQ)
        xq_t = np.ascontiguousarray(np.asarray(query[b, qsl], np.float32).T)
        xk_t = np.ascontiguousarray(np.asarray(key[b], np.float32).T.astype(bf))
        xv_t = np.ascontiguousarray(np.asarray(value[b], np.float32).T.astype(bf))
        in_maps.append({
            "xq32": xq_t, "xqb": np.ascontiguousarray(xq_t.astype(bf)),
            "xkb": xk_t, "xvb": xv_t, **shared,
        })
    return in_maps


def kernel(**inputs):
    nc = build()
    in_maps = make_in_maps(**inputs)
    res = run_bass_kernel_spmd(nc, in_maps, list(range(NCORES)))
    out = np.empty((B, S, D), np.float32)
    for core in range(NCORES):
        b, half = divmod(core, 2)
        out[b, half * TQ : (half + 1) * TQ] = res.results[core]["out"].T
    return out


if __name__ == "__main__":
    import reference

    inputs = {k: np.asarray(v) for k, v in reference.setup_inputs().items()}
    got = kernel(**inputs)
    exp = np.asarray(reference.reference(**inputs))
    err = np.abs(got - exp).max() / np.abs(exp).max()
    print("rel err:", err)


# revision 37
# speedup vs baseline: 1.4232x; 1.2180x over previous
"""Trainium2 Bass kernel for a cross-modal transformer block (attention + FFN).

Contract: kernel(**inputs) takes the FULL unsharded inputs (numpy, fp32) and
returns the FULL output [4, 2048, 512] fp32.

Sharding: 8 cores = data-parallel over batch (4) x query-sequence halves (2).
Each core computes K/V projections for its batch's full 2048-token sequence
(cheap duplication) so attention needs no collectives.

Device layout: everything feature-major ([features on partitions, tokens on
free]); the host pre-transposes and pre-casts inputs so the device does zero
transposes.
"""

import functools
import sys

import numpy as np

sys.path.insert(0, "/opt/trn_rl_repo")

import ml_dtypes  # noqa: E402

import concourse.bass as bass  # noqa: E402
import concourse.tile as tile  # noqa: E402
from concourse import bacc, mybir  # noqa: E402
from concourse.bass_utils import run_bass_kernel_spmd  # noqa: E402

BF16 = mybir.dt.bfloat16
F32 = mybir.dt.float32
AF = mybir.ActivationFunctionType
OP = mybir.AluOpType

B, S, D = 4, 2048, 512
H, DH = 8, 64
FF = 2048
P = 128
C = D // P  # 4 feature chunks
CF = FF // P  # 16 ffn chunks
TQ = S // 2  # 1024 query tokens per core
TK = S  # full key sequence per core
KC = TK // P  # 16 key chunks
NT = 512  # token tile (matmul free dim)
NQ = TQ // NT  # 2 query-token tiles
SCALE = 1.0 / np.sqrt(DH)  # 0.125
LN_EPS = 1e-5
NCORES = 8


def _emit(nc, t, es, tc):
    """Emit the per-core program. t: dict name -> DRAM AP."""
    # ---------------- pools ----------------
    pw = es.enter_context(tc.tile_pool(name="projw", bufs=1))
    wp = es.enter_context(tc.tile_pool(name="w", bufs=1))
    ap_ = es.enter_context(tc.tile_pool(name="acts", bufs=1))
    ptq = es.enter_context(tc.tile_pool(name="ptq", bufs=2))
    psS = es.enter_context(tc.tile_pool(name="psS", bufs=2, space="PSUM"))
    psC = es.enter_context(tc.tile_pool(name="psC", bufs=3, space="PSUM"))
    psE = es.enter_context(tc.tile_pool(name="psE", bufs=1, space="PSUM"))
    stream = es.enter_context(tc.tile_pool(name="stream", bufs=6))
    vpool = es.enter_context(tc.tile_pool(name="vpool", bufs=5))
    stage = es.enter_context(tc.tile_pool(name="stage", bufs=2))
    stage1 = es.enter_context(tc.tile_pool(name="stage1", bufs=1))
    chunk = es.enter_context(tc.tile_pool(name="chunk", bufs=2))
    chunk3 = es.enter_context(tc.tile_pool(name="chunk3", bufs=3))
    small = es.enter_context(tc.tile_pool(name="small", bufs=6))
    epool = es.enter_context(tc.tile_pool(name="e", bufs=6))
    hpool = es.enter_context(tc.tile_pool(name="h", bufs=1))

    def ld_w(pool, name, kchunks, n):
        w = pool.tile([P, kchunks, n], BF16, name=name + "_sb")
        src_ = t[name].rearrange("p (c o) -> p c o", c=kchunks)
        for ki in range(kchunks):
            nc.sync.dma_start(w[:, ki, :], src_[:, ki, :])
        return w

    # all small per-feature vectors arrive pre-shuffled in one [P, 48] pack
    ball = wp.tile([P, 48], F32, name="ball")
    nc.sync.dma_start(ball, t["ball"])
    bq, bk, bo, b2 = (ball[:, 4 * i : 4 * (i + 1)] for i in range(4))
    g1, be1, g2, be2 = (ball[:, 16 + 4 * i : 20 + 4 * i] for i in range(4))
    b1 = ball[:, 32:48]

    wk = ld_w(pw, "wk", C, D)
    kr0 = []
    for ki in range(C):
        r = stream.tile([P, 1024], BF16, tag="xr", name=f"kr_0_{ki}")
        nc.sync.dma_start(r[0:64], t["xkb"].rearrange("p (c q) -> p c q", c=C)[0:64, ki, 0:1024])
        nc.sync.dma_start(r[64:P], t["xkb"].rearrange("p (c q) -> p c q", c=C)[64:P, ki, 0:1024])
        kr0.append(r)
    wv = ld_w(pw, "wv", C, D)
    wq = ld_w(pw, "wq", C, D)
    bvb = pw.tile([P, D], F32)
    nc.sync.dma_start(bvb, t["bvb"])

    wo = ld_w(wp, "wo", C, D)
    w1d = t["w1"].rearrange("p (c o) -> p c o", c=C)
    w2d = t["w2"].rearrange("p (c o) -> p c o", c=CF)

    ones = wp.tile([P, 1], F32)
    nc.vector.memset(ones, 1.0)
    onesb = wp.tile([P, 1], BF16)
    nc.vector.memset(onesb, 1.0)
    epst = wp.tile([1, 1], F32)
    nc.vector.memset(epst, LN_EPS)

    # persistent activations (full key sequence); tags shared with the FFN
    # weights, which reuse these slots once attention is done
    kts = [ap_.tile([P, TK], BF16, tag=f"big1_{i}", name=f"kt_{i}") for i in range(C)]
    va = ap_.tile([P, KC, H, DH], BF16, tag="big2", name="va")  # V token-major

    xq32d = t["xq32"].rearrange("p (c q) -> p c q", c=C)
    xqbd = t["xqb"].rearrange("p (c q) -> p c q", c=C)
    xkb = t["xkb"].rearrange("p (c q) -> p c q", c=C)
    xvb = t["xvb"].rearrange("p (c q) -> p c q", c=C)
    out_d = t["out"].rearrange("(c p) q -> p c q", p=P)

    # ---------------- phase A: K/V/Q projections ----------------
    # inputs are loaded as [P, 1024] tiles (2KB DMA lines)
    for half in range(2):
        hs = slice(half * 1024, (half + 1) * 1024)
        if half == 0:
            kr = kr0
        else:
            kr = []
            for ki in range(C):
                r = stream.tile([P, 1024], BF16, tag="xr", name=f"kr_{half}_{ki}")
                nc.sync.dma_start(r[0:64], xkb[0:64, ki, hs])
                nc.sync.dma_start(r[64:P], xkb[64:P, ki, hs])
                kr.append(r)
        for tk in range(2):
            ts_ = slice(half * 1024 + tk * NT, half * 1024 + (tk + 1) * NT)
            tsl = slice(tk * NT, (tk + 1) * NT)
            for co in range(C):
                ps = psC.tile([P, NT], F32, tag="pc", name=f"kps_{half}_{tk}_{co}")
                for ki in range(C):
                    nc.tensor.matmul(
                        ps,
                        wk[:, ki, co * P : (co + 1) * P],
                        kr[ki][:, tsl],
                        start=(ki == 0),
                        stop=(ki == C - 1),
                    )
                nc.vector.tensor_scalar(
                    out=kts[co][:, ts_], in0=ps, scalar1=bk[:, co : co + 1],
                    scalar2=None, op0=OP.add,
                )

    # V projection, token-major out: V = Xv @ Wv  (lhsT = Xv.T chunk)
    for half in range(2):
        hs = slice(half * 1024, (half + 1) * 1024)
        vr = []
        for ki in range(C):
            r = stream.tile([P, 1024], BF16, tag="xr", name=f"vr_{half}_{ki}")
            nc.sync.dma_start(r[0:64], xvb[0:64, ki, hs])
            nc.sync.dma_start(r[64:P], xvb[64:P, ki, hs])
            vr.append(r)
        for tm8 in range(8):
            tm = half * 8 + tm8
            msl = slice(tm8 * P, (tm8 + 1) * P)
            ps = psC.tile([P, NT], F32, tag="pc", name=f"vps_{tm}")
            for ki in range(C):
                nc.tensor.matmul(
                    ps, vr[ki][:, msl], wv[:, ki, :],
                    start=(ki == 0), stop=(ki == C - 1),
                )
            nc.vector.tensor_tensor(
                out=va[:, tm, :, :],
                in0=ps.rearrange("p (h d) -> p h d", h=H),
                in1=bvb.rearrange("p (h d) -> p h d", h=H),
                op=OP.add,
            )

    # Q projections for both token tiles (so attention for either tile is
    # never blocked on projection work)
    qr = []
    for ki in range(C):
        r = stream.tile([P, TQ], BF16, tag="xr", name=f"qr_{ki}")
        nc.sync.dma_start(r[0:64], xqbd[0:64, ki, :])
        nc.sync.dma_start(r[64:P], xqbd[64:P, ki, :])
        qr.append(r)
    qts = []
    for tq in range(NQ):
        tsl = slice(tq * NT, (tq + 1) * NT)
        qt = ptq.tile([P, C, NT], BF16, tag="qt", name=f"qt_{tq}")
        for co in range(C):
            ps = psC.tile([P, NT], F32, tag="pc", name=f"qps_{tq}_{co}")
            for ki in range(C):
                nc.tensor.matmul(
                    ps,
                    wq[:, ki, co * P : (co + 1) * P],
                    qr[ki][:, tsl],
                    start=(ki == 0),
                    stop=(ki == C - 1),
                )
            nc.vector.tensor_scalar(
                out=qt[:, co, :], in0=ps, scalar1=bq[:, co : co + 1],
                scalar2=None, op0=OP.add,
            )
        qts.append(qt)

    # ---------------- phase B: attention (both token tiles) ----------------
    ctxs = []
    for tq in range(NQ):
        qt = qts[tq]
        ctx = ptq.tile([P, C, NT], BF16, tag="ctx", name=f"ctx_{tq}")
        for hp in range(H // 2):  # head pairs sharing a 128-partition chunk
            pc = [
                psC.tile([P, NT], F32, tag="pc", name=f"pc_{tq}_{hp}_{j}")
                for j in range(2)
            ]
            # software-pipelined: scores(kc)+exp(kc) emitted before ctx(kc-1)
            e2s = [None] * KC
            for kc in range(KC + 1):
                if kc < KC:
                    ksl = slice(kc * P, (kc + 1) * P)
                    ps2 = psS.tile(
                        [P, 2, NT], F32, tag="ps2", name=f"sps_{tq}_{hp}_{kc}"
                    )
                    e2 = epool.tile(
                        [P, 2, NT], BF16, tag="e", name=f"e_{tq}_{hp}_{kc}"
                    )
                    for j in range(2):  # head 2*hp + j at partition offset 64*j
                        rows = slice(j * DH, (j + 1) * DH)
                        # scores.T chunk = K_h @ Q_h.T
                        nc.tensor.matmul(
                            ps2[:, j, :], kt[rows, hp, ksl], qt[rows, hp, :],
                            start=True, stop=True,
                        )
                    nc.scalar.activation(e2, ps2, AF.Exp, scale=SCALE)
                    e2s[kc] = e2
                if kc >= 1:
                    for j in range(2):
                        # ctx.T (+ sumexp row 64): lhsT = [V_h | 1], rhs = E.T
                        nc.tensor.matmul(
                            pc[j][0 : DH + 1, :],
                            va[:, kc - 1, 2 * hp + j, :],
                            e2s[kc - 1][:, j, :],
                            start=(kc - 1 == 0),
                            stop=(kc - 1 == KC - 1),
                        )
            for j in range(2):
                # fast copies release the PSUM accumulator; approx reciprocal
                # (~4e-6 rel err, plenty for a softmax denominator) keeps the
                # DVE FIFO clear
                se = small.tile([1, NT], F32, tag="sm", name=f"se_{tq}_{hp}_{j}")
                nc.vector.tensor_copy(out=se, in_=pc[j][DH : DH + 1, :])
                cf = chunk.tile([DH, NT], F32, tag="cf", name=f"cf_{tq}_{hp}_{j}")
                nc.vector.tensor_copy(out=cf, in_=pc[j][0:DH, :])
                rc = small.tile([1, NT], F32, tag="sm", name=f"rc_{tq}_{hp}_{j}")
                nc.vector.reciprocal_approx_fast(out=rc, in_=se)
                db = chunk.tile([DH, NT], F32, tag="db", name=f"db_{tq}_{hp}_{j}")
                nc.gpsimd.partition_broadcast(db, rc)
                nc.vector.tensor_tensor(
                    out=ctx[j * DH : (j + 1) * DH, hp, :],
                    in0=cf,
                    in1=db,
                    op=OP.mult,
                )
        ctxs.append(ctx)

    # FFN weights reuse the kt/va slots (attention is done with them);
    # 4 DMAs each so the loads spread across queues and overlap Oproj/LN1
    w1s = ap_.tile([P, C, FF], BF16, tag="big1", name="w1s")
    for ki in range(C):
        nc.sync.dma_start(w1s[:, ki, :], w1d[:, ki, :])
    w2s = ap_.tile([P, CF, D], BF16, tag="big2", name="w2s")
    for kq in range(4):
        nc.sync.dma_start(w2s[:, 4 * kq : 4 * kq + 4, :], w2d[:, 4 * kq : 4 * kq + 4, :])

    # ---------------- phase C: tails, interleaved across token tiles ------
    def layernorm(resid, g, be, out_write, tag, out_write_co=None):
        """resid: [P, C, NT] f32 tile. out_write(co, t2_f32_tile, be_col)."""
        lnp = psC.tile([P, NT], F32, tag="pc", name=f"lnp_{tag}")
        for co in range(C):
            nc.tensor.matmul(lnp[0:1, :], ones, resid[:, co, :], start=(co == 0),
                             stop=(co == C - 1), skip_group_check=True)
        s4 = stage1.tile([P, C, NT], F32, tag="sq", name=f"sq_{tag}")
        nc.vector.tensor_mul(s4, resid, resid)
        for co in range(C):
            nc.tensor.matmul(lnp[64:65, :], ones, s4[:, co, :], start=(co == 0),
                             stop=(co == C - 1), tile_position=(0, 64),
                             skip_group_check=True)
        mean = small.tile([1, NT], F32, tag="sm", name=f"mean_{tag}")
        nc.vector.tensor_scalar_mul(mean, lnp[0:1, :], 1.0 / D)
        msq = small.tile([1, NT], F32, tag="sm", name=f"msq_{tag}")
        nc.vector.tensor_scalar_mul(msq, lnp[64:65, :], 1.0 / D)
        m2 = small.tile([1, NT], F32, tag="sm", name=f"m2_{tag}")
        nc.vector.tensor_mul(m2, mean, mean)
        var = small.tile([1, NT], F32, tag="sm", name=f"var_{tag}")
        nc.vector.tensor_tensor(out=var, in0=msq, in1=m2, op=OP.subtract)
        # rstd = exp(-0.5 * ln(var + eps)) -- stays in the Exp/Ln ACT table set
        lnv = small.tile([1, NT], F32, tag="sm", name=f"lnv_{tag}")
        nc.scalar.activation(lnv, var, AF.Ln, bias=epst)
        rstd = small.tile([1, NT], F32, tag="sm", name=f"rstd_{tag}")
        nc.scalar.activation(rstd, lnv, AF.Exp, scale=-0.5)
        meanb = chunk.tile([P, NT], F32, tag="bc", name=f"meanb_{tag}")
        nc.gpsimd.partition_broadcast(meanb, mean)
        rstdb = chunk.tile([P, NT], F32, tag="bc", name=f"rstdb_{tag}")
        nc.gpsimd.partition_broadcast(rstdb, rstd)
        tt = stage.tile([P, C, NT], F32, tag="lnt", name=f"tt_{tag}")
        nc.vector.tensor_tensor(
            out=tt, in0=resid,
            in1=meanb[:, None, :].to_broadcast((P, C, NT)), op=OP.subtract,
        )
        nc.vector.tensor_tensor(
            out=tt, in0=tt,
            in1=rstdb[:, None, :].to_broadcast((P, C, NT)), op=OP.mult,
        )
        nc.vector.tensor_tensor(
            out=tt, in0=tt,
            in1=g[:, :, None].to_broadcast((P, C, NT)), op=OP.mult,
        )
        out_write(tt, be)

    # O projection + residual (query + attn_out)
    resids = []
    for tq in range(NQ):
        ts_ = slice(tq * NT, (tq + 1) * NT)
        ctx = ctxs[tq]
        resid = stage.tile([P, C, NT], F32, tag="resid", name=f"resid_{tq}")
        for co in range(C):
            xqc = chunk.tile([P, NT], F32, tag="xqc", name=f"xqc_{tq}_{co}")
            nc.sync.dma_start(xqc, xq32d[:, co, ts_])
            ps = psC.tile([P, NT], F32, tag="pc", name=f"ops_{tq}_{co}")
            for ki in range(C):
                nc.tensor.matmul(
                    ps,
                    wo[:, ki, co * P : (co + 1) * P],
                    ctx[:, ki, :],
                    start=(ki == 0),
                    stop=(ki == C - 1),
                )
            nc.vector.scalar_tensor_tensor(
                out=resid[:, co, :],
                in0=ps,
                scalar=bo[:, co : co + 1],
                in1=xqc,
                op0=OP.add,
                op1=OP.add,
            )
        resids.append(resid)

    # LN1 for both tiles first (their chains overlap each other and the
    # Oproj tail), then per-tq FFN1/FFN2/LN2 (each LN2 chain overlaps the
    # next tile's FFN matmuls)
    ln1fs, ln1bs = [], []
    for tq in range(NQ):
        ln1f = stage.tile([P, C, NT], F32, tag="ln1f", name=f"ln1f_{tq}")
        ln1b = ptq.tile([P, C, NT], BF16, tag="ln1b", name=f"ln1b_{tq}")

        def write_ln1(tt, be, ln1f=ln1f, ln1b=ln1b):
            nc.vector.tensor_tensor(
                out=ln1f, in0=tt,
                in1=be[:, :, None].to_broadcast((P, C, NT)), op=OP.add,
            )
            nc.vector.tensor_copy(out=ln1b, in_=ln1f)

        layernorm(resids[tq], g1, be1, write_ln1, f"l1_{tq}")
        ln1fs.append(ln1f)
        ln1bs.append(ln1b)

    for tq in range(NQ):
        ts_ = slice(tq * NT, (tq + 1) * NT)
        hb = hpool.tile([P, CF, NT], BF16, tag="h", name=f"h_{tq}")
        for fo in range(CF):
            ps = psC.tile([P, NT], F32, tag="pc", name=f"fps_{tq}_{fo}")
            for ki in range(C):
                nc.tensor.matmul(
                    ps,
                    w1s[:, ki, fo * P : (fo + 1) * P],
                    ln1bs[tq][:, ki, :],
                    start=(ki == 0),
                    stop=(ki == C - 1),
                )
            nc.scalar.activation(hb[:, fo, :], ps, AF.Gelu, bias=b1[:, fo : fo + 1])

        resid2 = stage1.tile([P, C, NT], F32, tag="resid2", name=f"resid2_{tq}")
        for co in range(C):
            ps = psC.tile([P, NT], F32, tag="pc", name=f"gps_{tq}_{co}")
            for ki in range(CF):
                nc.tensor.matmul(
                    ps,
                    w2s[:, ki, co * P : (co + 1) * P],
                    hb[:, ki, :],
                    start=(ki == 0),
                    stop=(ki == CF - 1),
                )
            nc.vector.scalar_tensor_tensor(
                out=resid2[:, co, :],
                in0=ps,
                scalar=b2[:, co : co + 1],
                in1=ln1fs[tq][:, co, :],
                op0=OP.add,
                op1=OP.add,
            )

        def write_out(tt, be, ts_=ts_, tq=tq):
            oc = stage.tile([P, C, NT], F32, tag="lnt", name=f"oc_{tq}")
            nc.vector.tensor_tensor(
                out=oc, in0=tt,
                in1=be[:, :, None].to_broadcast((P, C, NT)), op=OP.add,
            )
            for co in range(C):
                nc.sync.dma_start(out_d[:, co, ts_], oc[:, co, :])

        layernorm(resid2, g2, be2, write_out, f"l2_{tq}")
